# revision 1
# baseline (speedup 1.0000x reference)
"""BoeNet kernel for 8 TRN2 NeuronCores (raw Bass, SPMD).

tokens -> embedding gather -> proj -> depth-2 greedy tree rollout
(policy gates p>=0.5 == [u>=0], child transform + sibling embeddings)
-> mean pool -> vocab projection (V=50257).
Output logits [4,1024,50257] fp32 (823MB) ==> memory-bound on the write.

Sharding: 2 position shards x 4 vocab shards = 8 cores.
Per core: tree compute for 2048 positions in fp32 (gates must be exact),
final projection in fp32r (~1.6e-4 rel err at full PE rate), logits
written as [2048, 12800] (vocab shard 12565 padded to 12800), trimmed on
host during unshard.

All compute stays in "transposed activation" layout XT = [h (2x128 part),
n] so matmuls contract over the partition dim.
"""

import contextlib

import numpy as np

import concourse.bass as bass
import concourse.mybir as mybir
from concourse.bass import IndirectOffsetOnAxis
from concourse.bass_utils import run_bass_kernel_spmd

F32 = mybir.dt.float32
F32R = mybir.dt.float32r
BF16 = mybir.dt.bfloat16
I32 = mybir.dt.int32
AF = mybir.ActivationFunctionType
OP = mybir.AluOpType

# problem constants
V, E, H = 50257, 256, 256
B, S = 4, 1024
NPOS = B * S
SIB_SCALE = float(1.0 / np.sqrt(H))

# sharding
P_SHARD, Q_SHARD = 2, 4

LAST_RESULT = None  # test.py inspects exec_time_ns here


def build_bass(npos_c, vc, v, nt=512, vgrp=5, reps=1, only=None, zero_bias=False):
    """Build the per-core SPMD program. npos_c positions, vc padded vocab."""
    T = npos_c // nt            # n-tiles
    NBLK = nt // 128            # p-blocks per n-tile
    NB = npos_c // 128
    NVT = vc // 512             # vocab tiles
    assert NVT % vgrp == 0
    NGRP = NVT // vgrp
    GCOLS = vgrp * 512

    nc = bass.Bass()
    cm = contextlib.ExitStack()

    # ---------------- DRAM parameters ----------------
    tok_in = nc.declare_dram_parameter("tok", [128, NB], I32, isOutput=False)
    emb_in = nc.declare_dram_parameter("emb", [v, E], F32, isOutput=False)
    projwt_in = nc.declare_dram_parameter("projwt", [128, 2, H], F32, isOutput=False)
    w1t_in = nc.declare_dram_parameter("w1t", [128, 2, H], F32, isOutput=False)
    wct_in = nc.declare_dram_parameter("wct", [128, 2, H], F32, isOutput=False)
    w2rep_in = nc.declare_dram_parameter("w2rep", [128, 2, 128], F32, isOutput=False)
    b1d_in = nc.declare_dram_parameter("b1d", [128, 2, 2], F32, isOutput=False)
    cb_in = nc.declare_dram_parameter("cb", [128, 2], F32, isOutput=False)
    pb_in = nc.declare_dram_parameter("pb", [128, 2], F32, isOutput=False)
    negb2_in = nc.declare_dram_parameter("negb2", [128, 1], F32, isOutput=False)
    sib_in = nc.declare_dram_parameter("sib", [128, 2, 2], F32, isOutput=False)
    sibsum_in = nc.declare_dram_parameter("sibsum", [128, 2], F32, isOutput=False)
    ident_in = nc.declare_dram_parameter("ident", [128, 128], F32, isOutput=False)
    outwt_in = nc.declare_dram_parameter("outwt", [128, 2, vc], F32R, isOutput=False)
    bias_in = nc.declare_dram_parameter("biasbc", [128, vc], BF16, isOutput=False)
    logits_out = nc.declare_dram_parameter("logits", [npos_c, vc], F32, isOutput=True)

    _n = [0]

    def sbuf(shape, dtype):
        _n[0] += 1
        return cm.enter_context(nc.sbuf_tensor(f"sb{_n[0]}", shape, dtype))

    def psum(shape):
        _n[0] += 1
        return cm.enter_context(nc.psum_tensor(f"ps{_n[0]}", shape, F32))

    # ---------------- SBUF ----------------
    tok_sb = sbuf([128, NB], I32)
    projwt = sbuf([128, 2, H], F32)
    w1t = sbuf([128, 2, H], F32)
    wct = sbuf([128, 2, H], F32)
    w2rep = sbuf([128, 2, 128], F32)
    b1d = sbuf([128, 2, 2], F32)
    cb = sbuf([128, 2], F32)
    pb = sbuf([128, 2], F32)
    negb2 = sbuf([128, 1], F32)
    sib = sbuf([128, 2, 2], F32)
    sibsum = sbuf([128, 2], F32)
    ident = sbuf([128, 128], F32)
    outwt = sbuf([128, 2, vc], F32R)
    bias_bc = sbuf([128, vc], BF16)

    g_sb = sbuf([128, 2 * NBLK, E], F32)
    embt = sbuf([128, 2, nt], F32)
    h0t = sbuf([128, 2, nt], F32)      # h0T; later doubles as the sum accumulator
    zt = sbuf([128, 2, nt], F32)       # z for current node; reused as base11
    base0 = sbuf([128, 2, nt], F32)
    base10 = sbuf([128, 2, nt], F32)
    c0t = sbuf([128, 2, nt], F32)
    c1t = sbuf([128, 2, nt], F32)
    suml = sbuf([128, 2, nt], F32)     # level-1 masked contribution
    g0 = sbuf([128, nt], F32)
    g10 = sbuf([128, nt], F32)
    g11 = sbuf([128, nt], F32)
    rec = g11                          # recip computed in place
    pooled = [sbuf([128, 2, nt], F32R) for _ in range(2)]
    res = [sbuf([128, GCOLS], F32) for _ in range(2)]

    ps_a = [psum([128, 512]) for _ in range(4)]
    ps_b = [psum([128, 512]) for _ in range(4)]

    dma_s = cm.enter_context(nc.semaphore("dma_s"))
    dma_g = cm.enter_context(nc.semaphore("dma_g"))
    pe_s = cm.enter_context(nc.semaphore("pe_s"))
    act_s = cm.enter_context(nc.semaphore("act_s"))
    dve_s = cm.enter_context(nc.semaphore("dve_s"))
    sems = {"dma_s": dma_s, "dma_g": dma_g, "pe": pe_s, "act": act_s, "dve": dve_s}

    cnt = {k: 0 for k in sems}
    prog = {"sync": [], "gpsimd": [], "tensor": [], "scalar": [], "vector": []}

    def emit(engine, fn):
        prog[engine].append(fn)

    last_wait = {}

    def wait(engine, sem_name, val):
        # skip waits already implied by an earlier wait on this engine+sem
        if val > 0 and last_wait.get((engine, sem_name), 0) < val:
            last_wait[(engine, sem_name)] = val
            emit(engine, lambda e, s=sems[sem_name], v=val: e.wait_ge(s, v))

    def tick(sem_name, n=1):
        cnt[sem_name] += n
        return cnt[sem_name]

    # ---------------- input DMAs ----------------
    def dma_in(dst, src):
        emit("sync", lambda e, dst=dst, src=src:
             e.dma_start(out=dst, in_=src).then_inc(dma_s, 16))
        return tick("dma_s", 16)

    for dst, src in [(projwt, projwt_in), (w1t, w1t_in), (wct, wct_in),
                     (w2rep, w2rep_in), (b1d, b1d_in), (cb, cb_in), (pb, pb_in),
                     (negb2, negb2_in), (sib, sib_in), (sibsum, sibsum_in),
                     (ident, ident_in), (bias_bc, bias_in),
                     (outwt, outwt_in)]:
        W_DONE = dma_in(dst[:], src[:])

    emit("gpsimd", lambda e: e.dma_start(out=tok_sb[:], in_=tok_in[:])
         .then_inc(dma_g, 16))
    TOK_DONE = tick("dma_g", 16)

    # psum B banks start free; phase B drains will wait on dve ticks
    bank_b_war = {k: ("dve", 0) for k in range(4)}

    # ---------------- WAR tick trackers ----------------
    bank_a_war = {k: ("act", 0) for k in range(4)}
    a_rr = [0]

    def a_bank():
        k = a_rr[0] % 4
        a_rr[0] += 1
        return k

    tr_pe_hist = {}          # t_glob -> pe tick after transposes of that tile
    prev = {
        "tr_pe": 0,          # PE done reading g_sb (transposes of tile t)
        "embt_pe": 0,        # PE done reading embt (proj MMs)
        "h0t_pe": 0,         # PE done reading h0t (z(n0)+child(n0) MMs)
        "base0_dve": 0,      # DVE done reading base0 (c_k + suml)
        "b10_dve": 0,        # DVE done reading base10 (t2_0)
        "b11_dve": 0,        # DVE done reading zt-as-base11 (t2_1)
        "c_pe": 0,           # PE done reading c0t/c1t
        "suml_dve": 0,
        "pooled_pe": 0,      # PE done reading pooled (phase B MMs)
        "pooled_dve": 0,     # DVE produced pooled / done reading h0t-sum
        "gates_dve": 0,
    }
    res_war = [("dma_s", 0), ("dma_s", 0)]
    res_rr = [0]
    b_rr = [0]
    pending_b = []
    pooled_pe_buf = [0, 0]

    def drain_pending(k):
        n = min(k, len(pending_b))
        for _ in range(n):
            pending_b.pop(0)()

    # fp32 matmul group over 2 K-halves into a phase-A bank
    def mm_group(lhsT_tile, mslice, rhs_tile, deps):
        bk = a_bank()
        s, v = bank_a_war[bk]
        wait("tensor", s, v)
        for ds, dv in deps:
            wait("tensor", ds, dv)
        for kh in range(2):
            stop = kh == 1
            emit("tensor", lambda e, bk=bk, kh=kh, lhsT_tile=lhsT_tile,
                 mslice=mslice, rhs_tile=rhs_tile, stop=stop:
                 (e.matmul(ps_a[bk][:], lhsT_tile[:, kh, mslice],
                           rhs_tile[:, kh, :], start=(not stop) if kh == 0 else False,
                           stop=stop).then_inc(pe_s, 1) if stop else
                  e.matmul(ps_a[bk][:], lhsT_tile[:, kh, mslice],
                           rhs_tile[:, kh, :], start=True, stop=False)))
        return bk, tick("pe")

    def _emit_phase_b(t, pbuf, pooled_ready):
        def make_group(t=t, pbuf=pbuf, pooled_ready=pooled_ready, i=None,
                       grp=None, is_last=False):
            def thunk():
                pos0 = t * nt + i * 128
                r = res_rr[0] % 2
                res_rr[0] += 1
                drain_start = True
                for jj in range(vgrp):
                    j = grp * vgrp + jj
                    bk = b_rr[0] % 4
                    b_rr[0] += 1
                    s, v = bank_b_war[bk]
                    wait("tensor", s, v)
                    wait("tensor", "dve", pooled_ready)
                    for kh in range(2):
                        stop = kh == 1
                        emit("tensor", lambda e, bk=bk, kh=kh, i=i, j=j, stop=stop,
                             pbuf=pbuf:
                             (e.matmul(ps_b[bk][:],
                                       pooled[pbuf][:, kh, 128 * i:128 * (i + 1)],
                                       outwt[:, kh, 512 * j:512 * (j + 1)],
                                       start=False, stop=True).then_inc(pe_s, 1)
                              if stop else
                              e.matmul(ps_b[bk][:],
                                       pooled[pbuf][:, kh, 128 * i:128 * (i + 1)],
                                       outwt[:, kh, 512 * j:512 * (j + 1)],
                                       start=True, stop=False)))
                    pt = tick("pe")
                    on_act = zero_bias and jj in (1, 3)
                    eng = "scalar" if on_act else "vector"
                    wait(eng, "pe", pt)
                    if drain_start:
                        ds, dv = res_war[r]
                        wait("vector", ds, dv)
                        wait("scalar", ds, dv)
                        drain_start = False
                    if on_act:
                        emit("scalar", lambda e, bk=bk, r=r, jj=jj:
                             e.activation(res[r][:, 512 * jj:512 * (jj + 1)],
                                          ps_b[bk][:], AF.Copy).then_inc(act_s, 1))
                        bank_b_war[bk] = ("act", tick("act"))
                    else:
                        emit("vector", lambda e, bk=bk, r=r, jj=jj, j=j:
                             e.tensor_tensor(out=res[r][:, 512 * jj:512 * (jj + 1)],
                                             in0=ps_b[bk][:],
                                             in1=bias_bc[:, 512 * j:512 * (j + 1)],
                                             op=OP.add).then_inc(dve_s, 1))
                        bank_b_war[bk] = ("dve", tick("dve"))
                wait("sync", "dve", cnt["dve"])
                if zero_bias:
                    wait("sync", "act", cnt["act"])
                emit("sync", lambda e, r=r, pos0=pos0, grp=grp:
                     e.dma_start(out=logits_out[pos0:pos0 + 128,
                                                GCOLS * grp:GCOLS * (grp + 1)],
                                 in_=res[r][:]).then_inc(dma_s, 16))
                res_war[r] = ("dma_s", tick("dma_s", 16))
                if is_last:
                    pooled_pe_buf[pbuf] = cnt["pe"]
            return thunk

        for i in range(NBLK):
            for grp in range(NGRP):
                pending_b.append(make_group(
                    i=i, grp=grp, is_last=(i == NBLK - 1 and grp == NGRP - 1)))

    for t_glob in range(T * reps):
        t = t_glob % T
        if t_glob > 0 and len(pending_b) > NBLK * NGRP:
            drain_pending(len(pending_b) - NBLK * NGRP)
        if only == "b":
            pbuf = t_glob % 2
            pooled_ready = 0
            _emit_phase_b(t, pbuf, pooled_ready)
            continue
        # ---- gathers (gpsimd): run 2 tiles ahead of the transposes ----
        wait("gpsimd", "dma_g", TOK_DONE)
        if t_glob >= 2 and tr_pe_hist.get(t_glob - 2):
            wait("gpsimd", "pe", tr_pe_hist[t_glob - 2])
        gat = {}
        for blk in range(NBLK):
            col = t * NBLK + blk
            slot = (t_glob * NBLK + blk) % (2 * NBLK)
            emit("gpsimd", lambda e, slot=slot, col=col:
                 e.indirect_dma_start(
                     out=g_sb[:, slot, :], out_offset=None, in_=emb_in[:, :],
                     in_offset=IndirectOffsetOnAxis(ap=tok_sb[:, col:col + 1], axis=0))
                 .then_inc(dma_g, 16))
            gat[blk] = tick("dma_g", 16)

        # ---- transposes (PE) + embt copies (ACT) ----
        embt_ready = 0
        for eh in range(2):
            bk = a_bank()
            s, v = bank_a_war[bk]
            wait("tensor", s, v)
            if t == 0 and eh == 0:
                wait("tensor", "dma_s", W_DONE)
            for blk in range(NBLK):
                wait("tensor", "dma_g", gat[blk])
                slot = (t_glob * NBLK + blk) % (2 * NBLK)
                stop = blk == NBLK - 1
                emit("tensor", lambda e, bk=bk, eh=eh, slot=slot, blk=blk, stop=stop:
                     (e.transpose(ps_a[bk][:, 128 * blk:128 * (blk + 1)],
                                  g_sb[:, slot, 128 * eh:128 * (eh + 1)], ident[:])
                      .then_inc(pe_s, 1) if stop else
                      e.transpose(ps_a[bk][:, 128 * blk:128 * (blk + 1)],
                                  g_sb[:, slot, 128 * eh:128 * (eh + 1)], ident[:])))
            pt = tick("pe")
            wait("scalar", "pe", pt)
            if eh == 0 and prev["embt_pe"]:
                wait("scalar", "pe", prev["embt_pe"])
            emit("scalar", lambda e, bk=bk, eh=eh:
                 e.activation(embt[:, eh, :], ps_a[bk][:], AF.Copy).then_inc(act_s, 1))
            bank_a_war[bk] = ("act", tick("act"))
        embt_ready = cnt["act"]
        prev["tr_pe"] = cnt["pe"]
        tr_pe_hist[t_glob] = cnt["pe"]
        drain_pending(3)

        # ---- proj -> h0t ----
        for m in range(2):
            bk, pt = mm_group(projwt, slice(128 * m, 128 * (m + 1)), embt,
                              [("act", embt_ready)])
            wait("scalar", "pe", pt)
            if m == 0 and prev["pooled_dve"]:
                wait("scalar", "dve", prev["pooled_dve"])  # h0t-as-sum free
            emit("scalar", lambda e, bk=bk, m=m:
                 e.activation(h0t[:, m, :], ps_a[bk][:], AF.Identity,
                              bias=pb[:, m:m + 1]).then_inc(act_s, 1))
            bank_a_war[bk] = ("act", tick("act"))
        h0_ready = cnt["act"]
        prev["embt_pe"] = cnt["pe"]
        drain_pending(3)

        # ---- policy: z=tanh(W1 x + b1d), u=w2.z, gate=[u>=-b2] ----
        def policy(x_tile, x_dep, depth, gate_out, zt_war):
            for m in range(2):
                bk, pt = mm_group(w1t, slice(128 * m, 128 * (m + 1)), x_tile,
                                  [x_dep])
                wait("scalar", "pe", pt)
                if m == 0 and zt_war[1]:
                    wait("scalar", zt_war[0], zt_war[1])
                emit("scalar", lambda e, bk=bk, m=m, depth=depth:
                     e.activation(zt[:, m, :], ps_a[bk][:], AF.Tanh,
                                  bias=b1d[:, m, depth:depth + 1]).then_inc(act_s, 1))
                bank_a_war[bk] = ("act", tick("act"))
            z_ready = cnt["act"]
            bk, pt = mm_group(w2rep, slice(0, 128), zt, [("act", z_ready)])
            wait("vector", "pe", pt)
            emit("vector", lambda e, bk=bk, gate_out=gate_out:
                 e.tensor_scalar(out=gate_out[:], in0=ps_a[bk][:],
                                 scalar1=negb2[:, 0:1], scalar2=None, op0=OP.is_ge)
                 .then_inc(dve_s, 1))
            bank_a_war[bk] = ("dve", tick("dve"))
            return cnt["dve"], pt  # gate tick (dve), zt free (pe tick)

        # ---- child: base = tanh(Wc x + cb) into base_tile ----
        def child(x_tile, x_dep, base_tile, base_war):
            for m in range(2):
                bk, pt = mm_group(wct, slice(128 * m, 128 * (m + 1)), x_tile,
                                  [x_dep])
                wait("scalar", "pe", pt)
                if m == 0 and base_war[1]:
                    wait("scalar", base_war[0], base_war[1])
                emit("scalar", lambda e, bk=bk, m=m, base_tile=base_tile:
                     e.activation(base_tile[:, m, :], ps_a[bk][:], AF.Tanh,
                                  bias=cb[:, m:m + 1]).then_inc(act_s, 1))
                bank_a_war[bk] = ("act", tick("act"))
            return cnt["act"]

        # node 0 (depth 0): zt WAR vs previous n-tile's base11 DVE reads
        g0_tick, zt_free = policy(h0t, ("act", h0_ready), 0, g0,
                                  ("dve", prev["b11_dve"]))
        base0_ready = child(h0t, ("act", h0_ready), base0,
                            ("dve", prev["base0_dve"]))
        prev["h0t_pe"] = cnt["pe"]
        drain_pending(3)

        # c_k = base0 + sib_k (DVE) ; also suml = c0 + c1 (= 2*base0+sibsum later * g0)
        wait("vector", "act", base0_ready)
        if prev["c_pe"]:
            wait("vector", "pe", prev["c_pe"])
        for m in range(2):
            emit("vector", lambda e, m=m:
                 e.tensor_scalar(out=c0t[:, m, :], in0=base0[:, m, :],
                                 scalar1=sib[:, m, 0:1], scalar2=None, op0=OP.add)
                 .then_inc(dve_s, 1))
            tick("dve")
            emit("vector", lambda e, m=m:
                 e.tensor_scalar(out=c1t[:, m, :], in0=base0[:, m, :],
                                 scalar1=sib[:, m, 1:2], scalar2=None, op0=OP.add)
                 .then_inc(dve_s, 1))
            tick("dve")
        c_ready = cnt["dve"]
        # suml = 2*base0 + sibsum (the unmasked level-1 sum) -- then *= g0 later
        for m in range(2):
            emit("vector", lambda e, m=m:
                 e.tensor_scalar(out=suml[:, m, :], in0=base0[:, m, :],
                                 scalar1=2.0, scalar2=sibsum[:, m:m + 1],
                                 op0=OP.mult, op1=OP.add).then_inc(dve_s, 1))
            tick("dve")
        prev["base0_dve"] = cnt["dve"]
        drain_pending(3)

        # node c0 (depth 1): zt WAR vs u(n0) MMs
        g10_tick, zt_free_c0 = policy(c0t, ("dve", c_ready), 1, g10,
                                      ("pe", zt_free))
        base10_ready = child(c0t, ("dve", c_ready), base10,
                             ("dve", prev["b10_dve"]))
        drain_pending(2)
        # node c1 (depth 1): zt WAR vs u(c0) MMs; then base11 goes into zt
        g11_tick, zt_free_c1 = policy(c1t, ("dve", c_ready), 1, g11,
                                      ("pe", zt_free_c0))
        base11_ready = child(c1t, ("dve", c_ready), zt,
                             ("pe", zt_free_c1))
        prev["c_pe"] = cnt["pe"]
        drain_pending(len(pending_b))

        # ---- DVE reduction chain (in-order on DVE) ----
        # masks: g10 *= g0 ; g11 *= g0
        emit("vector", lambda e: e.tensor_tensor(out=g10[:], in0=g10[:], in1=g0[:],
                                                 op=OP.mult).then_inc(dve_s, 1))
        tick("dve")
        emit("vector", lambda e: e.tensor_tensor(out=g11[:], in0=g11[:], in1=g0[:],
                                                 op=OP.mult).then_inc(dve_s, 1))
        tick("dve")
        # suml *= g0 ; h0t += suml
        for m in range(2):
            emit("vector", lambda e, m=m:
                 e.tensor_tensor(out=suml[:, m, :], in0=suml[:, m, :], in1=g0[:],
                                 op=OP.mult).then_inc(dve_s, 1))
            tick("dve")
            emit("vector", lambda e, m=m:
                 e.tensor_tensor(out=h0t[:, m, :], in0=h0t[:, m, :],
                                 in1=suml[:, m, :], op=OP.add).then_inc(dve_s, 1))
            tick("dve")
        prev["suml_dve"] = cnt["dve"]
        # t2_0 = (2*base10 + sibsum) * g10 ; h0t += t2_0   (base10 -> scratch suml)
        wait("vector", "act", base10_ready)
        for m in range(2):
            emit("vector", lambda e, m=m:
                 e.tensor_scalar(out=suml[:, m, :], in0=base10[:, m, :],
                                 scalar1=2.0, scalar2=sibsum[:, m:m + 1],
                                 op0=OP.mult, op1=OP.add).then_inc(dve_s, 1))
            tick("dve")
            emit("vector", lambda e, m=m:
                 e.tensor_tensor(out=suml[:, m, :], in0=suml[:, m, :], in1=g10[:],
                                 op=OP.mult).then_inc(dve_s, 1))
            tick("dve")
            emit("vector", lambda e, m=m:
                 e.tensor_tensor(out=h0t[:, m, :], in0=h0t[:, m, :],
                                 in1=suml[:, m, :], op=OP.add).then_inc(dve_s, 1))
            tick("dve")
        prev["b10_dve"] = cnt["dve"]
        # t2_1 = (2*base11(zt) + sibsum) * g11 ; h0t += t2_1
        wait("vector", "act", base11_ready)
        for m in range(2):
            emit("vector", lambda e, m=m:
                 e.tensor_scalar(out=suml[:, m, :], in0=zt[:, m, :],
                                 scalar1=2.0, scalar2=sibsum[:, m:m + 1],
                                 op0=OP.mult, op1=OP.add).then_inc(dve_s, 1))
            tick("dve")
            emit("vector", lambda e, m=m:
                 e.tensor_tensor(out=suml[:, m, :], in0=suml[:, m, :], in1=g11[:],
                                 op=OP.mult).then_inc(dve_s, 1))
            tick("dve")
            emit("vector", lambda e, m=m:
                 e.tensor_tensor(out=h0t[:, m, :], in0=h0t[:, m, :],
                                 in1=suml[:, m, :], op=OP.add).then_inc(dve_s, 1))
            tick("dve")
        prev["b11_dve"] = cnt["dve"]

        # cnt chain: g11 += g10 ; g11 += g0 ; g11 = 2*g11+1 ; rec = 1/g11
        emit("vector", lambda e: e.tensor_tensor(out=g11[:], in0=g11[:], in1=g10[:],
                                                 op=OP.add).then_inc(dve_s, 1))
        tick("dve")
        emit("vector", lambda e: e.tensor_tensor(out=g11[:], in0=g11[:], in1=g0[:],
                                                 op=OP.add).then_inc(dve_s, 1))
        tick("dve")
        emit("vector", lambda e: e.tensor_scalar(out=g11[:], in0=g11[:],
                                                 scalar1=2.0, scalar2=1.0,
                                                 op0=OP.mult, op1=OP.add)
             .then_inc(dve_s, 1))
        tick("dve")
        emit("vector", lambda e: e.reciprocal(out=rec[:], in_=g11[:])
             .then_inc(dve_s, 1))
        tick("dve")
        # pooled = h0t * rec  (f32r)
        pbuf = t_glob % 2
        if pooled_pe_buf[pbuf]:
            wait("vector", "pe", pooled_pe_buf[pbuf])
        for m in range(2):
            emit("vector", lambda e, m=m, pbuf=pbuf:
                 e.tensor_tensor(out=pooled[pbuf][:, m, :], in0=h0t[:, m, :],
                                 in1=rec[:], op=OP.mult).then_inc(dve_s, 1))
            tick("dve")
        pooled_ready = cnt["dve"]
        prev["pooled_dve"] = pooled_ready

        # ---------------- phase B: create thunks, interleaved into A(t+1) ----
        if only == "a":
            continue
        _emit_phase_b(t, pbuf, pooled_ready)



    drain_pending(len(pending_b))

    # final: ensure all DMAs complete before kernel end
    wait("sync", "dma_s", cnt["dma_s"])
    wait("gpsimd", "dma_g", cnt["dma_g"])

    # ---------------- emit engine blocks ----------------
    with nc.Block() as block:
        @block.sync
        def _(e):
            for fn in prog["sync"]:
                fn(e)

        @block.gpsimd
        def _(e):
            for fn in prog["gpsimd"]:
                fn(e)

        @block.tensor
        def _(e):
            for fn in prog["tensor"]:
                fn(e)

        @block.scalar
        def _(e):
            for fn in prog["scalar"]:
                fn(e)

        @block.vector
        def _(e):
            for fn in prog["vector"]:
                fn(e)

    nc._kernel_exitstack = cm  # keep SBUF/PSUM/semaphore contexts alive
    return nc


def _prep_weights(inputs, v, vc, q_shard):
    """Host-side input packing shared across cores."""
    f32 = np.float32
    emb = np.ascontiguousarray(np.asarray(inputs["embedding"], dtype=f32))
    proj_W = np.asarray(inputs["proj_W"], dtype=f32)
    proj_b = np.asarray(inputs["proj_b"], dtype=f32)
    child_W = np.asarray(inputs["child_W"], dtype=f32)
    child_b = np.asarray(inputs["child_b"], dtype=f32)
    sib_emb = np.asarray(inputs["sib_emb"], dtype=f32)
    depth_emb = np.asarray(inputs["depth_emb"], dtype=f32)
    pol_W1 = np.asarray(inputs["pol_W1"], dtype=f32)
    pol_b1 = np.asarray(inputs["pol_b1"], dtype=f32)
    pol_w2 = np.asarray(inputs["pol_w2"], dtype=f32)
    pol_b2 = np.asarray(inputs["pol_b2"], dtype=f32)
    out_W = np.asarray(inputs["out_W"], dtype=f32)
    out_b = np.asarray(inputs["out_b"], dtype=f32)

    def t_pack(w):  # [in, out] -> [128, 2, out]  (w.T reshaped)
        return np.ascontiguousarray(w.T.reshape(2, 128, w.shape[0]).transpose(1, 0, 2))

    common = {
        "emb": emb,
        "projwt": t_pack(proj_W),       # proj_W [H,E]: lhsT = proj_W.T [E,H]
        "w1t": t_pack(pol_W1),
        "wct": t_pack(child_W),
        "w2rep": np.ascontiguousarray(
            np.repeat(pol_w2.reshape(2, 128, 1).transpose(1, 0, 2), 128, axis=2)),
        "b1d": np.ascontiguousarray(
            (pol_b1[None, :] + depth_emb).T.reshape(2, 128, 2).transpose(1, 0, 2)),
        "cb": np.ascontiguousarray(child_b.reshape(2, 128).T),
        "pb": np.ascontiguousarray(proj_b.reshape(2, 128).T),
        "negb2": np.full((128, 1), -float(pol_b2), dtype=f32),
        "sib": np.ascontiguousarray(
            (SIB_SCALE * sib_emb).T.reshape(2, 128, 2).transpose(1, 0, 2)),
        "sibsum": np.ascontiguousarray(
            (SIB_SCALE * (sib_emb[0] + sib_emb[1])).reshape(2, 128).T),
        "ident": np.eye(128, dtype=f32),
        "ones": np.ones((1, 128), dtype=f32),
    }
    per_q = []
    for q in range(q_shard):
        lo = q * vc
        hi = min(lo + vc, v)
        wt = np.zeros((vc, H), dtype=f32)
        wt[:hi - lo] = out_W[lo:hi]
        ob = np.zeros((1, vc), dtype=f32)
        ob[0, :hi - lo] = out_b[lo:hi]
        import ml_dtypes
        per_q.append({
            "outwt": np.ascontiguousarray(
                wt.T.reshape(2, 128, vc).transpose(1, 0, 2)),
            "biasbc": np.ascontiguousarray(
                np.broadcast_to(ob.astype(ml_dtypes.bfloat16), (128, vc))),
        })
    return common, per_q


def _run_pjrt(nc, in_maps, n_cores=8, time_iters=0):
    """Execute via PJRT/shard_map (adapted from bass2jax.run_bass_via_pjrt,
    without donation so repeated timed calls are possible)."""
    import jax
    import numpy as _np
    from jax.sharding import Mesh, NamedSharding, PartitionSpec
    from jax.experimental.shard_map import shard_map

    from concourse import mybir as _mybir
    from concourse.bass2jax import (_bass_exec_p, install_neuronx_cc_hook,
                                    partition_id_tensor)

    install_neuronx_cc_hook()

    partition_name = (nc.partition_id_tensor.name
                      if nc.partition_id_tensor else None)
    in_names, out_names, out_avals = [], [], []
    for alloc in nc.m.functions[0].allocations:
        if not isinstance(alloc, _mybir.MemoryLocationSet):
            continue
        name = alloc.memorylocations[0].name
        if alloc.kind == "ExternalInput":
            if name == partition_name:
                continue
            in_names.append(name)
        elif alloc.kind == "ExternalOutput":
            out_names.append(name)
            out_avals.append(jax.core.ShapedArray(
                tuple(alloc.tensor_shape), _mybir.dt.np(alloc.dtype)))
    n_params = len(in_names)
    all_names = in_names + out_names
    if partition_name is not None:
        all_names = all_names + [partition_name]

    def _body(*args):
        operands = list(args)
        if partition_name is not None:
            operands.append(partition_id_tensor())
        outs = _bass_exec_p.bind(
            *operands,
            out_avals=tuple(out_avals),
            in_names=tuple(all_names),
            out_names=tuple(out_names),
            lowering_input_output_aliases=(),
            sim_require_finite=True,
            sim_require_nnan=True,
            nc=nc,
        )
        return tuple(outs)

    devices = jax.devices()[:n_cores]
    mesh = Mesh(_np.asarray(devices), ("core",))
    spec = PartitionSpec("core")
    n_outs = len(out_names)
    sharded = jax.jit(
        shard_map(_body, mesh=mesh, in_specs=(spec,) * (n_params + n_outs),
                  out_specs=(spec,) * n_outs, check_rep=False),
        keep_unused=True,
    )
    sh = NamedSharding(mesh, spec)
    dev_in = [
        jax.device_put(
            _np.concatenate([_np.asarray(in_maps[c][nm]) for c in range(n_cores)],
                            axis=0), sh)
        for nm in in_names
    ]
    dev_zero = [
        jax.device_put(
            _np.zeros((n_cores * a.shape[0], *a.shape[1:]), a.dtype), sh)
        for a in out_avals
    ]
    out = sharded(*dev_in, *dev_zero)
    jax.block_until_ready(out)
    exec_ns = None
    if time_iters:
        import time as _time
        times = []
        for _ in range(time_iters):
            t0 = _time.perf_counter()
            o2 = sharded(*dev_in, *dev_zero)
            jax.block_until_ready(o2)
            times.append(_time.perf_counter() - t0)
        exec_ns = int(min(times) * 1e9)
    results = [
        {nm: _np.asarray(out[i]).reshape(n_cores, *out_avals[i].shape)[c]
         for i, nm in enumerate(out_names)}
        for c in range(n_cores)
    ]
    return results, exec_ns


class _Result:
    def __init__(self, results, exec_time_ns):
        self.results = results
        self.exec_time_ns = exec_time_ns
        self.instructions_and_trace = None


def kernel(**inputs):
    global LAST_RESULT
    import os
    npos_c = NPOS // P_SHARD
    vc = 12800
    # zero_bias drain-split measured 2.3x SLOWER (ACT PSUM-copies ~2-9x DVE
    # and they block the phase-A tanh chain) -- keep the DVE bias-add drains.
    nc = build_bass(npos_c, vc, V, zero_bias=False)

    tokens = np.asarray(inputs["tokens"]).astype(np.int32).reshape(-1)
    common, per_q = _prep_weights(inputs, V, vc, Q_SHARD)

    in_maps = []
    for c in range(8):
        p, q = divmod(c, Q_SHARD)
        tok = tokens[p * npos_c:(p + 1) * npos_c]
        tok_pre = np.ascontiguousarray(tok.reshape(-1, 128).T)  # [128, NB]
        m = dict(common)
        m.update(per_q[q])
        m["tok"] = tok_pre
        in_maps.append(m)

    time_iters = int(os.environ.get("BASS_TIME_ITERS", "0"))
    results, exec_ns = _run_pjrt(nc, in_maps, n_cores=8, time_iters=time_iters)
    LAST_RESULT = _Result(results, exec_ns)

    full = np.empty((NPOS, V), dtype=np.float32)
    for c in range(8):
        p, q = divmod(c, Q_SHARD)
        lo = q * vc
        hi = min(lo + vc, V)
        full[p * npos_c:(p + 1) * npos_c, lo:hi] = \
            results[c]["logits"][:, :hi - lo]
    return full.reshape(B, S, V)



# revision 2
# speedup vs baseline: 3.3408x; 3.3408x over previous
"""BoeNet kernel for 8 TRN2 NeuronCores (raw Bass, SPMD) — v2.

tokens -> embedding gather -> proj -> depth-2 greedy tree rollout
(policy gates p>=0.5 == [u>=0], child transform + sibling embeddings)
-> mean pool -> vocab projection (V=50257).
Output logits [4,1024,50257]; written to HBM as bf16 (abs err ~2.6e-4 on a
0.132-scale output => rel ~2e-3, budget 2e-2), upcast to f32 on host.

Sharding: 4 position shards x 2 vocab shards = 8 cores. Per core: tree
compute for 1024 positions, vocab slice 25600 (padded from 25129).

Precision: gate margins in this instance are tiny (min |u1| = 1e-6; one
flipped gate costs rel err 0.38), so everything feeding the depth-1 gates
(proj, child@h0, W1@base0, w2@z1x) runs fp32 on the PE.  Depth-0 gates have
min margin 2.2e-4, and the last-level child transform feeds only the pooled
sum, so those paths can run f32r (full PE rate).

Phase A algebra: c_k = base0 + SIB*sib_k, so W1@c_k = W1@base0 + const_k and
Wc@c_k = Wc@base0 + const_k: sibling nodes share one matmul each, with the
per-sibling constants folded into the ACT bias port.  The masked tree sum is
  sum = h0 + 2*(g0*base0 + G10*b10 + G11*b11) + ss*q,   q = g0+G10+G11
  cnt = 1 + 2q
computed with fused scalar_tensor_tensor ops.

Phase B is "transposed": psum tiles are [128 vocab, 512 pos] (stationary =
bf16 out_W tile with FWL, moving = bf16 pooledT which phase A produces
natively).  The out_b bias becomes a per-partition scalar => drains are
single tensor_scalar/activation ops, split between DVE and ACT.  Logits land
in DRAM as [128, NVT, npos_c] bf16, unscrambled on host.
"""

import contextlib

import numpy as np

import concourse.bass as bass
import concourse.mybir as mybir
from concourse.bass import IndirectOffsetOnAxis
from concourse.bass_utils import run_bass_kernel_spmd  # noqa: F401  (env compat)

F32 = mybir.dt.float32
F32R = mybir.dt.float32r
BF16 = mybir.dt.bfloat16
I32 = mybir.dt.int32
AF = mybir.ActivationFunctionType
OP = mybir.AluOpType

# problem constants
V, E, H = 50257, 256, 256
B, S = 4, 1024
NPOS = B * S
SIB_SCALE = float(1.0 / np.sqrt(H))

# sharding
P_SHARD, Q_SHARD = 4, 2
VC = 25600           # padded per-core vocab slice (200 tiles of 128)

LAST_RESULT = None  # test.py inspects exec_time_ns here


def build_bass(npos_c, vc, v, nt=512, reps=1, only=None, act_every=4,
               cr_f32r=True, z0_f32r=False, dma_grp=8):
    """Per-core SPMD program. npos_c positions, vc padded vocab slice."""
    T = npos_c // nt            # n-tiles per rep
    NBLK = nt // 128            # gather blocks per n-tile
    NB = npos_c // 128
    NVT = vc // 128             # 128-row vocab tiles
    NCH = nt                    # phase B moving free size (positions chunk)
    assert NVT % dma_grp == 0
    NGRP = NVT // dma_grp

    nc = bass.Bass()
    cm = contextlib.ExitStack()

    # ---------------- DRAM parameters ----------------
    tok_in = nc.declare_dram_parameter("tok", [128, NB], I32, isOutput=False)
    emb_in = nc.declare_dram_parameter("emb", [v, E], F32, isOutput=False)
    projwt_in = nc.declare_dram_parameter("projwt", [128, 2, H], F32, isOutput=False)
    w1t_in = nc.declare_dram_parameter("w1t", [128, 2, H], F32, isOutput=False)
    wct_in = nc.declare_dram_parameter("wct", [128, 2, H], F32, isOutput=False)
    w2rep_in = nc.declare_dram_parameter("w2rep", [128, 2, 128], F32, isOutput=False)
    pb_in = nc.declare_dram_parameter("pb", [128, 2], F32, isOutput=False)
    cb_in = nc.declare_dram_parameter("cb", [128, 2], F32, isOutput=False)
    b1d0_in = nc.declare_dram_parameter("b1d0", [128, 2], F32, isOutput=False)
    zb_in = nc.declare_dram_parameter("zb", [128, 2, 2], F32, isOutput=False)
    cbk_in = nc.declare_dram_parameter("cbk", [128, 2, 2], F32, isOutput=False)
    negb2_in = nc.declare_dram_parameter("negb2", [128, 1], F32, isOutput=False)
    ss_in = nc.declare_dram_parameter("ss", [128, 2], F32, isOutput=False)
    ident_in = nc.declare_dram_parameter("ident", [128, 128], F32, isOutput=False)
    outwt_in = nc.declare_dram_parameter("outwt", [128, 2, vc], BF16, isOutput=False)
    outb_in = nc.declare_dram_parameter("outb", [128, NVT], F32, isOutput=False)
    if cr_f32r:
        wctr_in = nc.declare_dram_parameter("wctr", [128, 2, H], F32R,
                                            isOutput=False)
    if z0_f32r:
        w1tr_in = nc.declare_dram_parameter("w1tr", [128, 2, H], F32R,
                                            isOutput=False)
        w2repr_in = nc.declare_dram_parameter("w2repr", [128, 2, 128], F32R,
                                              isOutput=False)
    logits_out = nc.declare_dram_parameter("logits", [128, NVT, npos_c], BF16,
                                           isOutput=True)

    _n = [0]

    def sbuf(shape, dtype):
        _n[0] += 1
        return cm.enter_context(nc.sbuf_tensor(f"sb{_n[0]}", shape, dtype))

    def psum(shape):
        _n[0] += 1
        return cm.enter_context(nc.psum_tensor(f"ps{_n[0]}", shape, F32))

    # ---------------- SBUF ----------------
    tok_sb = sbuf([128, NB], I32)
    projwt = sbuf([128, 2, H], F32)
    w1t = sbuf([128, 2, H], F32)
    wct = sbuf([128, 2, H], F32)
    w2rep = sbuf([128, 2, 128], F32)
    pb = sbuf([128, 2], F32)
    cb = sbuf([128, 2], F32)
    b1d0 = sbuf([128, 2], F32)
    zb = sbuf([128, 2, 2], F32)
    cbk = sbuf([128, 2, 2], F32)
    negb2 = sbuf([128, 1], F32)
    ss = sbuf([128, 2], F32)
    ident = sbuf([128, 128], F32)
    outwt = sbuf([128, 2, vc], BF16)
    outb = sbuf([128, NVT], F32)
    wctr = sbuf([128, 2, H], F32R) if cr_f32r else None
    w1tr = sbuf([128, 2, H], F32R) if z0_f32r else None
    w2repr = sbuf([128, 2, 128], F32R) if z0_f32r else None

    g_sb = sbuf([128, 2 * NBLK, E], F32)
    embt = sbuf([128, 2, nt], F32)
    h0t = sbuf([128, 2, nt], F32)       # h0T; later the pooled-sum accumulator
    h0tr = sbuf([128, 2, nt], F32R) if z0_f32r else None
    base0 = sbuf([128, 2, nt], F32)
    base0r = sbuf([128, 2, nt], F32R) if cr_f32r else None
    z0 = sbuf([128, 2, nt], F32R if z0_f32r else F32)
    z10 = sbuf([128, 2, nt], F32)
    z11 = sbuf([128, 2, nt], F32)
    b10 = sbuf([128, 2, nt], F32)
    b11 = sbuf([128, 2, nt], F32)
    acc = sbuf([128, 2, nt], F32)
    tmp = sbuf([128, 2, nt], F32)
    g0 = sbuf([128, nt], F32)
    G10 = sbuf([128, nt], F32)
    G11 = sbuf([128, nt], F32)
    q = sbuf([128, nt], F32)
    pooled = [sbuf([128, 2, nt], BF16) for _ in range(2)]
    res = [sbuf([128, dma_grp, NCH], BF16) for _ in range(2)]

    ps_a = [psum([128, 512]) for _ in range(4)]
    ps_b = [psum([128, 512]) for _ in range(4)]

    dma_s = cm.enter_context(nc.semaphore("dma_s"))
    dma_g = cm.enter_context(nc.semaphore("dma_g"))
    pe_s = cm.enter_context(nc.semaphore("pe_s"))
    act_s = cm.enter_context(nc.semaphore("act_s"))
    dve_s = cm.enter_context(nc.semaphore("dve_s"))
    sems = {"dma_s": dma_s, "dma_g": dma_g, "pe": pe_s, "act": act_s,
            "dve": dve_s}

    cnt = {k: 0 for k in sems}
    prog = {"sync": [], "gpsimd": [], "tensor": [], "scalar": [], "vector": []}

    def emit(engine, fn):
        prog[engine].append(fn)

    last_wait = {}

    def wait(engine, sem_name, val):
        if val > 0 and last_wait.get((engine, sem_name), 0) < val:
            last_wait[(engine, sem_name)] = val
            emit(engine, lambda e, s=sems[sem_name], v=val: e.wait_ge(s, v))

    def tick(sem_name, n=1):
        cnt[sem_name] += n
        return cnt[sem_name]

    # ---------------- one-time input DMAs ----------------
    def dma_in(dst, src):
        emit("sync", lambda e, dst=dst, src=src:
             e.dma_start(out=dst, in_=src).then_inc(dma_s, 16))
        return tick("dma_s", 16)

    pairs = [(projwt, projwt_in), (w1t, w1t_in), (wct, wct_in),
             (w2rep, w2rep_in), (pb, pb_in), (cb, cb_in), (b1d0, b1d0_in),
             (zb, zb_in), (cbk, cbk_in), (negb2, negb2_in), (ss, ss_in),
             (ident, ident_in), (outb, outb_in), (outwt, outwt_in)]
    if cr_f32r:
        pairs.append((wctr, wctr_in))
    if z0_f32r:
        pairs += [(w1tr, w1tr_in), (w2repr, w2repr_in)]
    for dst, src in pairs:
        W_DONE = dma_in(dst[:], src[:])

    emit("gpsimd", lambda e: e.dma_start(out=tok_sb[:], in_=tok_in[:])
         .then_inc(dma_g, 16))
    TOK_DONE = tick("dma_g", 16)

    # ---------------- WAR tick trackers ----------------
    bank_a_war = {k: ("act", 0) for k in range(4)}
    bank_b_war = {k: ("dve", 0) for k in range(4)}
    a_rr = [0]

    def a_bank():
        k = a_rr[0] % 4
        a_rr[0] += 1
        return k

    tr_pe_hist = {}
    prev = {
        "embt_pe": 0,     # PE done reading embt (proj MMs)
        "h0t_pe": 0,      # PE done reading h0t/h0tr (W1@h0 + Wc@h0 MMs)
        "base0_pe": 0,    # PE done reading base0[r] (W1@base0 + Wc@base0)
        "base0_dve": 0,   # DVE done reading base0 (acc chain)
        "z0_pe": 0,       # PE done reading z0 (w2@z0)
        "z1x_pe": 0,      # PE done reading z10/z11
        "b1x_dve": 0,     # DVE done reading b10/b11
        "pooled_dve": 0,  # DVE produced pooled / finished reading h0t
    }
    res_war = [("dma_s", 0), ("dma_s", 0)]
    pooled_pe_buf = [0, 0]
    b_rr = [0]
    res_rr = [0]
    pending_b = []

    def drain_pending(k):
        n = min(k, len(pending_b))
        for _ in range(n):
            pending_b.pop(0)()

    # fp32/f32r matmul group: accumulate 2 K-halves into one phase-A bank
    def mm_group(lhsT_tile, mslice, rhs_tile, deps):
        bk = a_bank()
        s, val = bank_a_war[bk]
        wait("tensor", s, val)
        for ds, dv in deps:
            wait("tensor", ds, dv)
        for kh in range(2):
            stop = kh == 1
            emit("tensor", lambda e, bk=bk, kh=kh, lhsT_tile=lhsT_tile,
                 mslice=mslice, rhs_tile=rhs_tile, stop=stop:
                 (e.matmul(ps_a[bk][:], lhsT_tile[:, kh, mslice],
                           rhs_tile[:, kh, :], start=False,
                           stop=True).then_inc(pe_s, 1) if stop else
                  e.matmul(ps_a[bk][:], lhsT_tile[:, kh, mslice],
                           rhs_tile[:, kh, :], start=True, stop=False)))
        return bk, tick("pe")

    # ---------------- phase B (transposed) ----------------
    def _emit_phase_b(t, pbuf, pooled_ready):
        chunk = t  # position chunk == tile index within the rep

        def vtile_thunk(vt, is_last):
            def thunk():
                bk = b_rr[0] % 4
                b_rr[0] += 1
                s, val = bank_b_war[bk]
                wait("tensor", s, val)
                wait("tensor", "dve", pooled_ready)
                for kh in range(2):
                    stop = kh == 1
                    emit("tensor", lambda e, bk=bk, kh=kh, vt=vt, stop=stop,
                         pbuf=pbuf:
                         (e.matmul(ps_b[bk][:],
                                   outwt[:, kh, 128 * vt:128 * (vt + 1)],
                                   pooled[pbuf][:, kh, :],
                                   start=False, stop=True).then_inc(pe_s, 1)
                          if stop else
                          e.matmul(ps_b[bk][:],
                                   outwt[:, kh, 128 * vt:128 * (vt + 1)],
                                   pooled[pbuf][:, kh, :],
                                   start=True, stop=False)))
                pt = tick("pe")
                r = res_rr[0] % 2
                slot = vt % dma_grp
                on_act = act_every and (vt % act_every == act_every - 1)
                eng = "scalar" if on_act else "vector"
                wait(eng, "pe", pt)
                if slot == 0:
                    ds, dv = res_war[r]
                    wait("vector", ds, dv)
                    wait("scalar", ds, dv)
                if on_act:
                    emit("scalar", lambda e, bk=bk, r=r, slot=slot, vt=vt:
                         e.activation(res[r][:, slot, :], ps_b[bk][:],
                                      AF.Identity, bias=outb[:, vt:vt + 1])
                         .then_inc(act_s, 1))
                    bank_b_war[bk] = ("act", tick("act"))
                else:
                    emit("vector", lambda e, bk=bk, r=r, slot=slot, vt=vt:
                         e.tensor_scalar(out=res[r][:, slot, :],
                                         in0=ps_b[bk][:],
                                         scalar1=outb[:, vt:vt + 1],
                                         scalar2=None, op0=OP.add)
                         .then_inc(dve_s, 1))
                    bank_b_war[bk] = ("dve", tick("dve"))
                if slot == dma_grp - 1:
                    grp = vt // dma_grp
                    wait("sync", "dve", cnt["dve"])
                    wait("sync", "act", cnt["act"])
                    emit("sync", lambda e, r=r, grp=grp, chunk=chunk:
                         e.dma_start(
                             out=logits_out[:,
                                            dma_grp * grp:dma_grp * (grp + 1),
                                            NCH * chunk:NCH * (chunk + 1)],
                             in_=res[r][:]).then_inc(dma_s, 16))
                    res_war[r] = ("dma_s", tick("dma_s", 16))
                    res_rr[0] += 1
                if is_last:
                    pooled_pe_buf[pbuf] = cnt["pe"]
            return thunk

        for vt in range(NVT):
            pending_b.append(vtile_thunk(vt, vt == NVT - 1))

    # ---------------- main loop ----------------
    for t_glob in range(T * reps):
        t = t_glob % T
        if t_glob > 0 and len(pending_b) > NVT:
            drain_pending(len(pending_b) - NVT)
        if only == "b":
            _emit_phase_b(t, t_glob % 2, 0)
            continue

        # ---- gathers (gpsimd): 2 tiles ahead of the transposes ----
        wait("gpsimd", "dma_g", TOK_DONE)
        if t_glob >= 2 and tr_pe_hist.get(t_glob - 2):
            wait("gpsimd", "pe", tr_pe_hist[t_glob - 2])
        gat = {}
        for blk in range(NBLK):
            col = t * NBLK + blk
            slot = (t_glob * NBLK + blk) % (2 * NBLK)
            emit("gpsimd", lambda e, slot=slot, col=col:
                 e.indirect_dma_start(
                     out=g_sb[:, slot, :], out_offset=None, in_=emb_in[:, :],
                     in_offset=IndirectOffsetOnAxis(ap=tok_sb[:, col:col + 1],
                                                    axis=0))
                 .then_inc(dma_g, 16))
            gat[blk] = tick("dma_g", 16)

        # ---- transposes (PE) + embt copies (ACT) ----
        for eh in range(2):
            bk = a_bank()
            s, val = bank_a_war[bk]
            wait("tensor", s, val)
            if t_glob == 0 and eh == 0:
                wait("tensor", "dma_s", W_DONE)
            for blk in range(NBLK):
                wait("tensor", "dma_g", gat[blk])
                slot = (t_glob * NBLK + blk) % (2 * NBLK)
                stop = blk == NBLK - 1
                emit("tensor", lambda e, bk=bk, eh=eh, slot=slot, blk=blk,
                     stop=stop:
                     (e.transpose(ps_a[bk][:, 128 * blk:128 * (blk + 1)],
                                  g_sb[:, slot, 128 * eh:128 * (eh + 1)],
                                  ident[:])
                      .then_inc(pe_s, 1) if stop else
                      e.transpose(ps_a[bk][:, 128 * blk:128 * (blk + 1)],
                                  g_sb[:, slot, 128 * eh:128 * (eh + 1)],
                                  ident[:])))
            pt = tick("pe")
            wait("scalar", "pe", pt)
            if eh == 0 and prev["embt_pe"]:
                wait("scalar", "pe", prev["embt_pe"])
            emit("scalar", lambda e, bk=bk, eh=eh:
                 e.activation(embt[:, eh, :], ps_a[bk][:], AF.Copy)
                 .then_inc(act_s, 1))
            bank_a_war[bk] = ("act", tick("act"))
        embt_ready = cnt["act"]
        tr_pe_hist[t_glob] = cnt["pe"]
        drain_pending(26)

        # ---- proj -> h0t (+h0tr) ----
        for m in range(2):
            bk, pt = mm_group(projwt, slice(128 * m, 128 * (m + 1)), embt,
                              [("act", embt_ready)])
            wait("scalar", "pe", pt)
            if m == 0:
                if prev["pooled_dve"]:
                    wait("scalar", "dve", prev["pooled_dve"])
                if prev["h0t_pe"]:
                    wait("scalar", "pe", prev["h0t_pe"])
            emit("scalar", lambda e, bk=bk, m=m:
                 e.activation(h0t[:, m, :], ps_a[bk][:], AF.Identity,
                              bias=pb[:, m:m + 1]).then_inc(act_s, 1))
            tick("act")
            if z0_f32r:
                emit("scalar", lambda e, bk=bk, m=m:
                     e.activation(h0tr[:, m, :], ps_a[bk][:], AF.Identity,
                                  bias=pb[:, m:m + 1]).then_inc(act_s, 1))
                tick("act")
            bank_a_war[bk] = ("act", cnt["act"])
        h0_ready = cnt["act"]
        prev["embt_pe"] = cnt["pe"]
        drain_pending(26)

        # ---- Wc@h0 -> base0 (+base0r) ----
        for m in range(2):
            bk, pt = mm_group(wct, slice(128 * m, 128 * (m + 1)), h0t,
                              [("act", h0_ready)])
            wait("scalar", "pe", pt)
            if m == 0 and prev["base0_dve"]:
                wait("scalar", "dve", prev["base0_dve"])
            if m == 0 and prev["base0_pe"]:
                wait("scalar", "pe", prev["base0_pe"])
            emit("scalar", lambda e, bk=bk, m=m:
                 e.activation(base0[:, m, :], ps_a[bk][:], AF.Tanh,
                              bias=cb[:, m:m + 1]).then_inc(act_s, 1))
            tick("act")
            if cr_f32r:
                emit("scalar", lambda e, bk=bk, m=m:
                     e.activation(base0r[:, m, :], ps_a[bk][:], AF.Tanh,
                                  bias=cb[:, m:m + 1]).then_inc(act_s, 1))
                tick("act")
            bank_a_war[bk] = ("act", cnt["act"])
        base0_ready = cnt["act"]
        drain_pending(26)

        # ---- W1@h0 -> z0 ; u0 = w2@z0 ; g0 ----
        w1_h0 = (w1tr, h0tr) if z0_f32r else (w1t, h0t)
        for m in range(2):
            bk, pt = mm_group(w1_h0[0], slice(128 * m, 128 * (m + 1)),
                              w1_h0[1], [("act", h0_ready)])
            wait("scalar", "pe", pt)
            if m == 0 and prev["z0_pe"]:
                wait("scalar", "pe", prev["z0_pe"])
            emit("scalar", lambda e, bk=bk, m=m:
                 e.activation(z0[:, m, :], ps_a[bk][:], AF.Tanh,
                              bias=b1d0[:, m:m + 1]).then_inc(act_s, 1))
            tick("act")
            bank_a_war[bk] = ("act", cnt["act"])
        z0_ready = cnt["act"]
        prev["h0t_pe"] = cnt["pe"]
        drain_pending(13)
        bk, pt = mm_group(w2repr if z0_f32r else w2rep, slice(0, 128), z0,
                          [("act", z0_ready)])
        prev["z0_pe"] = cnt["pe"]
        wait("vector", "pe", pt)
        emit("vector", lambda e, bk=bk:
             e.tensor_scalar(out=g0[:], in0=ps_a[bk][:],
                             scalar1=negb2[:, 0:1], scalar2=None,
                             op0=OP.is_ge).then_inc(dve_s, 1))
        bank_a_war[bk] = ("dve", tick("dve"))
        drain_pending(13)

        # ---- W1@base0 -> z10, z11 (shared matmul, sibling biases) ----
        for m in range(2):
            bk, pt = mm_group(w1t, slice(128 * m, 128 * (m + 1)), base0,
                              [("act", base0_ready)])
            wait("scalar", "pe", pt)
            if m == 0 and prev["z1x_pe"]:
                wait("scalar", "pe", prev["z1x_pe"])
            for (k, ztile) in ((0, z10), (1, z11)):
                emit("scalar", lambda e, bk=bk, m=m, k=k, ztile=ztile:
                     e.activation(ztile[:, m, :], ps_a[bk][:], AF.Tanh,
                                  bias=zb[:, m, k:k + 1]).then_inc(act_s, 1))
                tick("act")
            bank_a_war[bk] = ("act", cnt["act"])
        z1x_ready = cnt["act"]
        drain_pending(26)

        # ---- u10 = w2@z10 -> G10 = [u10>=thr]*g0 ; same for z11 ----
        for (ztile, Gt) in ((z10, G10), (z11, G11)):
            bk, pt = mm_group(w2rep, slice(0, 128), ztile,
                              [("act", z1x_ready)])
            wait("vector", "pe", pt)
            emit("vector", lambda e, bk=bk, Gt=Gt:
                 e.scalar_tensor_tensor(out=Gt[:], in0=ps_a[bk][:],
                                        scalar=negb2[:, 0:1], in1=g0[:],
                                        op0=OP.is_ge, op1=OP.mult)
                 .then_inc(dve_s, 1))
            bank_a_war[bk] = ("dve", tick("dve"))
        prev["z1x_pe"] = cnt["pe"]
        drain_pending(13)

        # ---- Wc@base0 -> b10, b11 (shared matmul, sibling biases) ----
        wc_b = (wctr, base0r) if cr_f32r else (wct, base0)
        for m in range(2):
            bk, pt = mm_group(wc_b[0], slice(128 * m, 128 * (m + 1)),
                              wc_b[1], [("act", base0_ready)])
            wait("scalar", "pe", pt)
            if m == 0 and prev["b1x_dve"]:
                wait("scalar", "dve", prev["b1x_dve"])
            for (k, btile) in ((0, b10), (1, b11)):
                emit("scalar", lambda e, bk=bk, m=m, k=k, btile=btile:
                     e.activation(btile[:, m, :], ps_a[bk][:], AF.Tanh,
                                  bias=cbk[:, m, k:k + 1]).then_inc(act_s, 1))
                tick("act")
            bank_a_war[bk] = ("act", cnt["act"])
        b1x_ready = cnt["act"]
        prev["base0_pe"] = cnt["pe"]
        drain_pending(13)

        # ---- DVE reduction: sum = h0 + 2(g0 b0 + G10 b10 + G11 b11) + ss q
        wait("vector", "act", base0_ready)
        for m in range(2):
            emit("vector", lambda e, m=m:
                 e.scalar_tensor_tensor(out=acc[:, m, :], in0=base0[:, m, :],
                                        scalar=2.0, in1=g0[:],
                                        op0=OP.mult, op1=OP.mult)
                 .then_inc(dve_s, 1))
            tick("dve")
        prev["base0_dve"] = cnt["dve"]
        wait("vector", "act", b1x_ready)
        for (btile, Gt) in ((b10, G10), (b11, G11)):
            for m in range(2):
                emit("vector", lambda e, m=m, btile=btile, Gt=Gt:
                     e.scalar_tensor_tensor(out=tmp[:, m, :],
                                            in0=btile[:, m, :], scalar=2.0,
                                            in1=Gt[:], op0=OP.mult,
                                            op1=OP.mult).then_inc(dve_s, 1))
                tick("dve")
                emit("vector", lambda e, m=m:
                     e.tensor_tensor(out=acc[:, m, :], in0=acc[:, m, :],
                                     in1=tmp[:, m, :], op=OP.add)
                     .then_inc(dve_s, 1))
                tick("dve")
        prev["b1x_dve"] = cnt["dve"]
        # q = g0 + G10 + G11 ; h0t += acc ; h0t += ss*q
        emit("vector", lambda e: e.tensor_tensor(out=q[:], in0=G10[:],
                                                 in1=g0[:], op=OP.add)
             .then_inc(dve_s, 1))
        tick("dve")
        emit("vector", lambda e: e.tensor_tensor(out=q[:], in0=q[:],
                                                 in1=G11[:], op=OP.add)
             .then_inc(dve_s, 1))
        tick("dve")
        for m in range(2):
            emit("vector", lambda e, m=m:
                 e.tensor_tensor(out=h0t[:, m, :], in0=h0t[:, m, :],
                                 in1=acc[:, m, :], op=OP.add)
                 .then_inc(dve_s, 1))
            tick("dve")
            emit("vector", lambda e, m=m:
                 e.scalar_tensor_tensor(out=h0t[:, m, :], in0=q[:],
                                        scalar=ss[:, m:m + 1],
                                        in1=h0t[:, m, :],
                                        op0=OP.mult, op1=OP.add)
                 .then_inc(dve_s, 1))
            tick("dve")
        # rec = 1/(2q+1) ; pooled = h0t*rec (bf16)
        emit("vector", lambda e: e.tensor_scalar(out=q[:], in0=q[:],
                                                 scalar1=2.0, scalar2=1.0,
                                                 op0=OP.mult, op1=OP.add)
             .then_inc(dve_s, 1))
        tick("dve")
        emit("vector", lambda e: e.reciprocal(out=q[:], in_=q[:])
             .then_inc(dve_s, 1))
        tick("dve")
        pbuf = t_glob % 2
        if pooled_pe_buf[pbuf]:
            wait("vector", "pe", pooled_pe_buf[pbuf])
        for m in range(2):
            emit("vector", lambda e, m=m, pbuf=pbuf:
                 e.tensor_tensor(out=pooled[pbuf][:, m, :], in0=h0t[:, m, :],
                                 in1=q[:], op=OP.mult).then_inc(dve_s, 1))
            tick("dve")
        pooled_ready = cnt["dve"]
        prev["pooled_dve"] = pooled_ready

        if only == "a":
            continue
        _emit_phase_b(t, pbuf, pooled_ready)

    drain_pending(len(pending_b))

    wait("sync", "dma_s", cnt["dma_s"])
    wait("gpsimd", "dma_g", cnt["dma_g"])

    # ---------------- emit engine blocks ----------------
    with nc.Block() as block:
        @block.sync
        def _(e):
            for fn in prog["sync"]:
                fn(e)

        @block.gpsimd
        def _(e):
            for fn in prog["gpsimd"]:
                fn(e)

        @block.tensor
        def _(e):
            for fn in prog["tensor"]:
                fn(e)

        @block.scalar
        def _(e):
            for fn in prog["scalar"]:
                fn(e)

        @block.vector
        def _(e):
            for fn in prog["vector"]:
                fn(e)

    nc._kernel_exitstack = cm
    return nc


def _prep_weights(inputs, v, vc, q_shard):
    """Host-side input packing shared across cores."""
    import ml_dtypes
    f32 = np.float32
    f64 = np.float64
    emb = np.ascontiguousarray(np.asarray(inputs["embedding"], dtype=f32))
    proj_W = np.asarray(inputs["proj_W"], dtype=f32)
    proj_b = np.asarray(inputs["proj_b"], dtype=f64)
    child_W = np.asarray(inputs["child_W"], dtype=f32)
    child_b = np.asarray(inputs["child_b"], dtype=f64)
    sib_emb = np.asarray(inputs["sib_emb"], dtype=f64)
    depth_emb = np.asarray(inputs["depth_emb"], dtype=f64)
    pol_W1 = np.asarray(inputs["pol_W1"], dtype=f32)
    pol_b1 = np.asarray(inputs["pol_b1"], dtype=f64)
    pol_w2 = np.asarray(inputs["pol_w2"], dtype=f32)
    pol_b2 = np.asarray(inputs["pol_b2"], dtype=f64)
    out_W = np.asarray(inputs["out_W"], dtype=f32)
    out_b = np.asarray(inputs["out_b"], dtype=f32)

    def t_pack(w):  # [out, in] -> [128, 2, out]  (lhsT = w.T packed)
        return np.ascontiguousarray(
            w.T.reshape(2, 128, w.shape[0]).transpose(1, 0, 2))

    def v_pack(x):  # [H] -> [128, 2]
        return np.ascontiguousarray(np.asarray(x, dtype=f32).reshape(2, 128).T)

    # sibling-folded biases (f64 precompute)
    sib_s = SIB_SCALE * sib_emb                       # [K, H]
    zb = np.stack([pol_b1 + depth_emb[1]
                   + pol_W1.astype(f64) @ sib_s[k] for k in range(2)],
                  axis=1)                             # [H, K]
    cbk = np.stack([child_b
                    + child_W.astype(f64) @ sib_s[k] for k in range(2)],
                   axis=1)                            # [H, K]

    common = {
        "emb": emb,
        "projwt": t_pack(proj_W),
        "w1t": t_pack(pol_W1),
        "wct": t_pack(child_W),
        "w2rep": np.ascontiguousarray(
            np.repeat(pol_w2.reshape(2, 128, 1).transpose(1, 0, 2), 128,
                      axis=2)),
        "pb": v_pack(proj_b),
        "cb": v_pack(child_b),
        "b1d0": v_pack(pol_b1 + depth_emb[0]),
        "zb": np.ascontiguousarray(
            zb.astype(f32).reshape(2, 128, 2).transpose(1, 0, 2)),
        "cbk": np.ascontiguousarray(
            cbk.astype(f32).reshape(2, 128, 2).transpose(1, 0, 2)),
        "negb2": np.full((128, 1), -float(pol_b2), dtype=f32),
        "ss": v_pack(sib_s[0] + sib_s[1]),
        "ident": np.eye(128, dtype=f32),
        "wctr": t_pack(child_W),
        "w1tr": t_pack(pol_W1),
        "w2repr": None,  # filled below
    }
    common["w2repr"] = common["w2rep"]
    nvt = vc // 128
    per_q = []
    for qi in range(q_shard):
        lo = qi * vc
        hi = min(lo + vc, v)
        wt = np.zeros((vc, H), dtype=f32)
        wt[:hi - lo] = out_W[lo:hi]
        ob = np.zeros((vc,), dtype=f32)
        ob[:hi - lo] = out_b[lo:hi]
        per_q.append({
            "outwt": np.ascontiguousarray(
                wt.T.reshape(2, 128, vc).transpose(1, 0, 2)
                .astype(ml_dtypes.bfloat16)),
            "outb": np.ascontiguousarray(ob.reshape(nvt, 128).T),
        })
    return common, per_q


def _run_pjrt(nc, in_maps, n_cores=8, time_iters=0):
    """Execute via PJRT/shard_map (adapted from bass2jax.run_bass_via_pjrt,
    without donation so repeated timed calls are possible)."""
    import jax
    import numpy as _np
    from jax.sharding import Mesh, NamedSharding, PartitionSpec
    from jax.experimental.shard_map import shard_map

    from concourse import mybir as _mybir
    from concourse.bass2jax import (_bass_exec_p, install_neuronx_cc_hook,
                                    partition_id_tensor)

    install_neuronx_cc_hook()

    partition_name = (nc.partition_id_tensor.name
                      if nc.partition_id_tensor else None)
    in_names, out_names, out_avals = [], [], []
    for alloc in nc.m.functions[0].allocations:
        if not isinstance(alloc, _mybir.MemoryLocationSet):
            continue
        name = alloc.memorylocations[0].name
        if alloc.kind == "ExternalInput":
            if name == partition_name:
                continue
            in_names.append(name)
        elif alloc.kind == "ExternalOutput":
            out_names.append(name)
            out_avals.append(jax.core.ShapedArray(
                tuple(alloc.tensor_shape), _mybir.dt.np(alloc.dtype)))
    n_params = len(in_names)
    all_names = in_names + out_names
    if partition_name is not None:
        all_names = all_names + [partition_name]

    def _body(*args):
        operands = list(args)
        if partition_name is not None:
            operands.append(partition_id_tensor())
        outs = _bass_exec_p.bind(
            *operands,
            out_avals=tuple(out_avals),
            in_names=tuple(all_names),
            out_names=tuple(out_names),
            lowering_input_output_aliases=(),
            sim_require_finite=True,
            sim_require_nnan=True,
            nc=nc,
        )
        return tuple(outs)

    devices = jax.devices()[:n_cores]
    mesh = Mesh(_np.asarray(devices), ("core",))
    spec = PartitionSpec("core")
    n_outs = len(out_names)
    sharded = jax.jit(
        shard_map(_body, mesh=mesh, in_specs=(spec,) * (n_params + n_outs),
                  out_specs=(spec,) * n_outs, check_rep=False),
        keep_unused=True,
    )
    sh = NamedSharding(mesh, spec)
    dev_in = [
        jax.device_put(
            _np.concatenate([_np.asarray(in_maps[c][nm]) for c in range(n_cores)],
                            axis=0), sh)
        for nm in in_names
    ]
    dev_zero = [
        jax.device_put(
            _np.zeros((n_cores * a.shape[0], *a.shape[1:]), a.dtype), sh)
        for a in out_avals
    ]
    out = sharded(*dev_in, *dev_zero)
    jax.block_until_ready(out)
    exec_ns = None
    if time_iters:
        import time as _time
        times = []
        for _ in range(time_iters):
            t0 = _time.perf_counter()
            o2 = sharded(*dev_in, *dev_zero)
            jax.block_until_ready(o2)
            times.append(_time.perf_counter() - t0)
        exec_ns = int(min(times) * 1e9)
    results = [
        {nm: _np.asarray(out[i]).reshape(n_cores, *out_avals[i].shape)[c]
         for i, nm in enumerate(out_names)}
        for c in range(n_cores)
    ]
    return results, exec_ns


class _Result:
    def __init__(self, results, exec_time_ns):
        self.results = results
        self.exec_time_ns = exec_time_ns
        self.instructions_and_trace = None


def make_in_maps(inputs, n_cores=8):
    npos_c = NPOS // P_SHARD
    tokens = np.asarray(inputs["tokens"]).astype(np.int32).reshape(-1)
    common, per_q = _prep_weights(inputs, V, VC, Q_SHARD)
    in_maps = []
    for c in range(n_cores):
        p, qi = divmod(c, Q_SHARD)
        tok = tokens[p * npos_c:(p + 1) * npos_c]
        m = dict(common)
        m.update(per_q[qi])
        m["tok"] = np.ascontiguousarray(tok.reshape(-1, 128).T)
        in_maps.append(m)
    return in_maps


def kernel(**inputs):
    global LAST_RESULT
    import os
    npos_c = NPOS // P_SHARD
    nc = build_bass(npos_c, VC, V)

    in_maps = make_in_maps(inputs)
    time_iters = int(os.environ.get("BASS_TIME_ITERS", "0"))
    results, exec_ns = _run_pjrt(nc, in_maps, n_cores=8, time_iters=time_iters)
    LAST_RESULT = _Result(results, exec_ns)

    full = np.empty((NPOS, V), dtype=np.float32)
    for c in range(8):
        p, qi = divmod(c, Q_SHARD)
        lo = qi * VC
        hi = min(lo + VC, V)
        lg = np.asarray(results[c]["logits"]).astype(np.float32)
        # [128, NVT, npos_c] -> [npos_c, NVT*128] (vocab = 128*vt + partition)
        lg = lg.transpose(2, 1, 0).reshape(npos_c, VC)
        full[p * npos_c:(p + 1) * npos_c, lo:hi] = lg[:, :hi - lo]
    return full.reshape(B, S, V)
